# revision 1
# baseline (speedup 1.0000x reference)
"""Trainium2 Bass kernel for nn_Butterfly_1580547970089.

Butterfly multiply (n=1024, log_n=10, nstacks=nblocks=1) + bias over a
16384-row batch, data-parallel across 8 NeuronCores (2048 rows each).

Decomposition (per core, features on partitions, batch on the free dim):
  * Stages 0-6 (strides 1..64) only mix features within 128-blocks; they are
    composed on the host into eight dense 128x128 matrices A_h.
  * Stage 7 (stride 128) pairs adjacent 128-blocks with per-feature 2x2
    coefficients; it is FOLDED into the matmuls: out-tile p of pair (p,q) is
    diag(c00)A_p x_p + diag(c01)A_q x_q, accumulated in PSUM.
  * Each folded matrix is applied with a 3-term bf16 split (W_h x_h + W_h x_e
    + W_e x_h, PSUM accumulates in fp32), giving ~2^-18 relative accuracy at
    bf16 matmul speed. The hi/lo bf16 input planes are prepared on the host,
    so DMA-in bytes equal the fp32 input (8 MB/core).
  * Stages 8 and 9 mix across 128-blocks at strides 256/512 with
    per-partition scalar coefficients: ACT computes the scaled temp
    (activation scale= / bias= per-partition APs, bias of stage 9 fused),
    DVE finishes with scalar_tensor_tensor. Stage 8 reads directly from PSUM
    (doubling as the PSUM evacuation).
  * The feature-major (transposed) layout is produced/consumed on the host as
    part of sharding, so the device does no transposes at all.

Device per core:
  in: xT [2048 rows, 2048 cols] bf16 — row block (ci*1024 + g*128 + p) holds
      feature f=g*128+p of batch chunk ci; cols 0:1024 = hi plane, 1024:2048 = lo.
  weights At [128, 32*128] bf16: per out-tile t: 4 blocks (Bh, Be, Ch, Ce),
      each the TRANSPOSED 128x128 matrix (lhsT layout).
  coef [128, 64] fp32: cols 16..31 s8 coeffs, 32..47 s9, 48..55 bias.
  out: outT [2048, 1024] fp32, same row-block layout, cols = chunk batch.
"""
import numpy as np
import ml_dtypes

import concourse.mybir as mybir
import concourse.tile as tile
from concourse import bacc, bass_utils

F32 = mybir.dt.float32
BF16 = mybir.dt.bfloat16
MULT = mybir.AluOpType.mult
ADD = mybir.AluOpType.add

N_CORES = 8
BATCH = 16384
N = 1024
B_CORE = BATCH // N_CORES
CHUNK = 1024
N_CHUNKS = B_CORE // CHUNK

S7_PAIRS = [(0, 1), (2, 3), (4, 5), (6, 7)]
S8_PAIRS = [(0, 2), (4, 6), (1, 3), (5, 7)]
S9_PAIRS = [(0, 4), (2, 6), (1, 5), (3, 7)]

_compiled = {}


def _emit_kernel(loop_reps=None):
    nc = bacc.Bacc("TRN2", target_bir_lowering=False, debug=False)
    xT = nc.dram_tensor("xT", [N * N_CHUNKS, 2 * CHUNK], BF16,
                        kind="ExternalInput").ap()
    At = nc.dram_tensor("At", [128, 32 * 128], BF16, kind="ExternalInput").ap()
    coef = nc.dram_tensor("coef", [128, 64], F32, kind="ExternalInput").ap()
    outT = nc.dram_tensor("outT", [N * N_CHUNKS, CHUNK], F32,
                          kind="ExternalOutput").ap()

    with tile.TileContext(nc) as tc:
        with (
            tc.tile_pool(name="const", bufs=1) as cpool,
            tc.tile_pool(name="xin", bufs=16) as xpool,
            tc.tile_pool(name="zbuf", bufs=16) as zpool,
            tc.tile_pool(name="tmp", bufs=12) as tpool,
            tc.tile_pool(name="y1", bufs=8, space="PSUM") as ppool,
        ):
            at = cpool.tile([128, 32 * 128], BF16, tag="at")
            nc.sync.dma_start(at[:], At[:])
            cf = cpool.tile([128, 64], F32, tag="cf")
            nc.sync.dma_start(cf[:], coef[:])

            def c(col):
                return cf[:, col:col + 1]

            def w(t, k):
                off = (t * 4 + k) * 128
                return at[:, off:off + 128]

            def chunk_body(ci):
                xt = [None] * 8
                for g in range(8):
                    xt[g] = xpool.tile([128, 2 * CHUNK], BF16, tag="xt",
                                       name=f"xt{g}")
                    row = ci * N + g * 128
                    nc.sync.dma_start(xt[g][:], xT[row:row + 128, :])
                z = [None] * 8
                for g in range(8):
                    z[g] = zpool.tile([128, CHUNK], F32, tag="z", name=f"z{g}")
                for sub in range(0, CHUNK, 512):
                    ps = {}
                    for (p_, q_) in S7_PAIRS:
                        for out_t, main, oth in ((p_, p_, q_), (q_, q_, p_)):
                            pt = ppool.tile([128, 512], F32, tag="y1",
                                            name=f"ps{out_t}")
                            xh_m = xt[main][:, sub:sub + 512]
                            xe_m = xt[main][:, CHUNK + sub:CHUNK + sub + 512]
                            xh_o = xt[oth][:, sub:sub + 512]
                            xe_o = xt[oth][:, CHUNK + sub:CHUNK + sub + 512]
                            nc.tensor.matmul(pt[:], w(out_t, 0), xh_m,
                                             start=True, stop=False)
                            nc.tensor.matmul(pt[:], w(out_t, 0), xe_m,
                                             start=False, stop=False)
                            nc.tensor.matmul(pt[:], w(out_t, 1), xh_m,
                                             start=False, stop=False)
                            nc.tensor.matmul(pt[:], w(out_t, 2), xh_o,
                                             start=False, stop=False)
                            nc.tensor.matmul(pt[:], w(out_t, 2), xe_o,
                                             start=False, stop=False)
                            nc.tensor.matmul(pt[:], w(out_t, 3), xh_o,
                                             start=False, stop=True)
                            ps[out_t] = pt
                    zs = [z[g][:, sub:sub + 512] for g in range(8)]
                    # stage 8: evacuates PSUM
                    for (p_, q_) in S8_PAIRS:
                        base = 16 + 4 * {(0,2):0, (1,3):1, (4,6):2, (5,7):3}[(p_, q_)]
                        u, v = ps[p_], ps[q_]
                        t1 = tpool.tile([128, 512], F32, tag="tmp")
                        nc.scalar.mul(t1[:], v[:], c(base + 1))
                        t2 = tpool.tile([128, 512], F32, tag="tmp")
                        nc.scalar.mul(t2[:], u[:], c(base + 2))
                        nc.vector.scalar_tensor_tensor(
                            zs[p_], u[:], c(base + 0), t1[:], op0=MULT, op1=ADD)
                        nc.vector.scalar_tensor_tensor(
                            zs[q_], v[:], c(base + 3), t2[:], op0=MULT, op1=ADD)
                    # stage 9 with fused bias
                    for (p_, q_) in S9_PAIRS:
                        base = 32 + 4 * {(0,4):0, (1,5):1, (2,6):2, (3,7):3}[(p_, q_)]
                        u, v = zs[p_], zs[q_]
                        t1 = tpool.tile([128, 512], F32, tag="tmp")
                        nc.scalar.activation(
                            t1[:], v, mybir.ActivationFunctionType.Identity,
                            bias=c(48 + p_), scale=c(base + 1))
                        t2 = tpool.tile([128, 512], F32, tag="tmp")
                        nc.scalar.activation(
                            t2[:], u, mybir.ActivationFunctionType.Identity,
                            bias=c(48 + q_), scale=c(base + 2))
                        nc.vector.scalar_tensor_tensor(
                            u, u, c(base + 0), t1[:], op0=MULT, op1=ADD)
                        nc.vector.scalar_tensor_tensor(
                            v, v, c(base + 3), t2[:], op0=MULT, op1=ADD)
                    for g in range(8):
                        row = ci * N + g * 128
                        e = nc.scalar if g % 2 == 0 else nc.sync
                        e.dma_start(
                            outT[row:row + 128, sub:sub + 512],
                            z[g][:, sub:sub + 512])

            def body():
                for ci in range(N_CHUNKS):
                    chunk_body(ci)

            if loop_reps is not None:
                with tc.For_i(0, loop_reps, 1,
                              hint_engines=(mybir.EngineType.PE,
                                            mybir.EngineType.DVE,
                                            mybir.EngineType.Activation)):
                    body()
            else:
                body()

    nc.compile()
    return nc


def _get_compiled(loop_reps=None):
    if loop_reps not in _compiled:
        _compiled[loop_reps] = _emit_kernel(loop_reps)
    return _compiled[loop_reps]


def _build_A(twiddle):
    A = np.zeros((8, 128, 128), np.float64)
    for h in range(8):
        M = np.eye(128, dtype=np.float64)
        for idx in range(7):
            s = 1 << idx
            tw = twiddle[0, 0, idx].astype(np.float64).reshape(512 // s, s, 2, 2)
            tw_h = tw[h * (64 // s):(h + 1) * (64 // s)]
            Mv = M.reshape(64 // s, 2, s, 128)
            top, bot = Mv[:, 0], Mv[:, 1]
            M = np.stack(
                [tw_h[:, :, 0, 0][..., None] * top + tw_h[:, :, 0, 1][..., None] * bot,
                 tw_h[:, :, 1, 0][..., None] * top + tw_h[:, :, 1, 1][..., None] * bot],
                axis=1).reshape(128, 128)
        A[h] = M
    return A


def _split_bf16(M):
    hi = M.astype(ml_dtypes.bfloat16)
    lo = (M - hi.astype(np.float64)).astype(ml_dtypes.bfloat16)
    return hi, lo


def _build_weights(twiddle):
    """At [128, 32*128] bf16: per out-tile t: [Bh, Be, Ch, Ce] transposed."""
    A = _build_A(twiddle)
    t7 = twiddle[0, 0, 7].reshape(4, 128, 2, 2).astype(np.float64)
    At = np.zeros((128, 32 * 128), ml_dtypes.bfloat16)
    for gi, (p, q) in enumerate(S7_PAIRS):
        pairs = [
            (p, np.diag(t7[gi, :, 0, 0]) @ A[p], np.diag(t7[gi, :, 0, 1]) @ A[q]),
            (q, np.diag(t7[gi, :, 1, 1]) @ A[q], np.diag(t7[gi, :, 1, 0]) @ A[p]),
        ]
        for out_t, B, C in pairs:
            Bh, Be = _split_bf16(B.T)
            Ch, Ce = _split_bf16(C.T)
            for k, M in enumerate((Bh, Be, Ch, Ce)):
                off = (out_t * 4 + k) * 128
                At[:, off:off + 128] = M
    return At


def _build_coef(twiddle, bias):
    coef = np.zeros((128, 64), np.float32)
    t8 = twiddle[0, 0, 8].reshape(2, 256, 2, 2)
    t9 = twiddle[0, 0, 9].reshape(1, 512, 2, 2)
    for gi in range(4):
        G, p = divmod(gi, 2)
        sl = slice(p * 128, (p + 1) * 128)
        for k, (i, j) in enumerate([(0, 0), (0, 1), (1, 0), (1, 1)]):
            coef[:, 16 + 4 * gi + k] = t8[G, sl, i, j]
    for p in range(4):
        sl = slice(p * 128, (p + 1) * 128)
        for k, (i, j) in enumerate([(0, 0), (0, 1), (1, 0), (1, 1)]):
            coef[:, 32 + 4 * p + k] = t9[0, sl, i, j]
    coef[:, 48:56] = bias.reshape(8, 128).T
    return coef


def _build_xT(shard):
    """shard [B_CORE, 1024] fp32 -> [N*N_CHUNKS, 2*CHUNK] bf16 blocked layout."""
    out = np.empty((N * N_CHUNKS, 2 * CHUNK), ml_dtypes.bfloat16)
    for ci in range(N_CHUNKS):
        blk = shard[ci * CHUNK:(ci + 1) * CHUNK, :].T  # [1024 f, CHUNK b]
        hi = blk.astype(ml_dtypes.bfloat16)
        lo = (blk - hi.astype(np.float32)).astype(ml_dtypes.bfloat16)
        out[ci * N:(ci + 1) * N, 0:CHUNK] = hi
        out[ci * N:(ci + 1) * N, CHUNK:2 * CHUNK] = lo
    return out


def kernel(input, twiddle, bias):
    input = np.asarray(input)
    twiddle = np.asarray(twiddle)
    bias = np.asarray(bias)
    nc = _get_compiled()

    At = _build_weights(twiddle)
    coef = _build_coef(twiddle, bias)
    in_maps = []
    for cid in range(N_CORES):
        shard = input[cid * B_CORE:(cid + 1) * B_CORE, :]
        in_maps.append({"xT": _build_xT(shard), "At": At, "coef": coef})

    res = bass_utils.run_bass_kernel_spmd(nc, in_maps,
                                          core_ids=list(range(N_CORES)))
    out = np.empty((BATCH, N), np.float32)
    for cid in range(N_CORES):
        o = res.results[cid]["outT"]  # [N*N_CHUNKS, CHUNK]
        for ci in range(N_CHUNKS):
            out[cid * B_CORE + ci * CHUNK:cid * B_CORE + (ci + 1) * CHUNK, :] = \
                o[ci * N:(ci + 1) * N, :].T
    return out

